# revision 1
# baseline (speedup 1.0000x reference)
"""Trainium2 Bass kernel for DifferentiableExtrusion.

Takes FULL inputs (B=8 batch), shards batch across 8 NeuronCores (1 b per
core), runs a Tile-scheduled Bass kernel per core, gathers full output.

Per-core algorithm (V=64 grid, N=32 polygons, P=16 vertices):
  distance: for each polygon edge, squared point-segment distance via the
    projection-clamp diff form (dx^2+dy^2), min-accumulated over edges.
    Layout: partitions = (mb*32+n) where mb = quarter of the 4096 grid
    points, free = 1024 points per quarter. Per-edge constants live in
    per-partition scalar columns so everything is tensor_scalar /
    scalar_tensor_tensor / ACT-activation ops.
  inside test: exact ray-cast parity done at (n,p,y) granularity (32k elems):
    intersection x is quantized to a 64-bit column mask (2 int32 words),
    masks XOR-reduced over edges -> per-(n,y) parity words, broadcast back
    to the grid and tested against a per-column bit mask.
  combine: mask = |inside - sigmoid(-100*d)|, gated by polygon validity,
    max-reduced over n (gpsimd partition_all_reduce), extruded over D=64
    slices with the per-b height mask, DMA'd out.
"""

import numpy as np
from contextlib import ExitStack

import concourse.bass as bass
import concourse.bacc as bacc
import concourse.mybir as mybir
import concourse.tile as tile
import concourse.bass_isa as bass_isa
from concourse.bass_utils import run_bass_kernel_spmd

f32 = mybir.dt.float32
i32 = mybir.dt.int32
A = mybir.AluOpType
AF = mybir.ActivationFunctionType
RO = bass_isa.ReduceOp

V = 64
N = 32
P = 16
M = V * V          # 4096 grid points
F = M // 4         # 1024 points per partition-quarter
SHARP = 100.0
EPS = 1e-8
BIGNEG = -1.0e30

# engine-assignment knobs (tuned via HW experiments)
W_ENGINE = "vector"
Z_ENGINE = "gpsimd"
DPOOL_BUFS = 4


def _make_consts():
    grid = (np.arange(V, dtype=np.float32) / np.float32(V - 1.0)).astype(np.float32)
    f = np.arange(F)
    x_of_f = f % V                      # x index for free position f
    fy_of_f = f // V                    # y-within-quarter for free position f
    mb = np.arange(128) // 32
    PX = np.broadcast_to(grid[x_of_f], (128, F)).astype(np.float32)
    PY = grid[(mb[:, None] * 16 + fy_of_f[None, :])].astype(np.float32)
    XB = np.broadcast_to((np.int32(1) << (x_of_f % 16).astype(np.int32)), (128, F)).astype(np.int32)
    YG = np.broadcast_to(grid, (128, V)).astype(np.float32)
    DP1 = (np.arange(V, dtype=np.float32) + 1.0).reshape(V, 1).astype(np.float32)
    E0 = np.zeros((V, 1), np.float32)
    E0[0, 0] = 1.0
    return PX, PY, XB, YG, DP1, E0


def _build_program(repeat=None):
    nc = bacc.Bacc("TRN2", target_bir_lowering=False, debug=False)

    polygons = nc.dram_tensor("polygons", [N, P, 2], f32, kind="ExternalInput")
    attributes = nc.dram_tensor("attributes", [6], f32, kind="ExternalInput")
    validity = nc.dram_tensor("validity_scores", [N], f32, kind="ExternalInput")
    out = nc.dram_tensor("out", [V, V, V], f32, kind="ExternalOutput")

    PXn, PYn, XBn, YGn, DP1n, E0n = _make_consts()
    PX_d = nc.inline_tensor(PXn, "PX_c")
    PY_d = nc.inline_tensor(PYn, "PY_c")
    XB_d = nc.inline_tensor(XBn, "XB_c")
    YG_d = nc.inline_tensor(YGn, "YG_c")
    DP1_d = nc.inline_tensor(DP1n, "DP1_c")
    E0_d = nc.inline_tensor(E0n, "E0_c")

    # DRAM scratch
    pw1_d = nc.dram_tensor("pw1_scratch", [128, 4 * V], i32)    # pre-fold parity words
    pw_d = nc.dram_tensor("pw_scratch", [N, V, 4], i32)       # folded parity words [n, y, w]
    comb_d = nc.dram_tensor("comb_scratch", [M], f32)         # combined mask, flat

    pg = polygons[:].rearrange("n p t -> n (p t)")            # [32, 32] DRAM view

    with tile.TileContext(nc) as tc, ExitStack() as ctx:
        cpool = ctx.enter_context(tc.tile_pool(name="consts", bufs=1))
        spool = ctx.enter_context(tc.tile_pool(name="setup", bufs=1))
        xpool = ctx.enter_context(tc.tile_pool(name="cross", bufs=1))
        dpool = ctx.enter_context(tc.tile_pool(name="dist", bufs=DPOOL_BUFS))
        apool = ctx.enter_context(tc.tile_pool(name="accum", bufs=1))
        fpool = ctx.enter_context(tc.tile_pool(name="final", bufs=1))

        # ---------------- constants into SBUF ----------------
        PX = cpool.tile([128, F], f32)
        nc.scalar.dma_start(PX[:], PX_d[:])
        PY = cpool.tile([128, F], f32)
        nc.gpsimd.dma_start(PY[:], PY_d[:])
        XB = cpool.tile([128, F], i32)
        nc.scalar.dma_start(XB[:], XB_d[:])
        YG = cpool.tile([128, V], f32)
        nc.gpsimd.dma_start(YG[:], YG_d[:])
        DP1 = cpool.tile([V, 1], f32)
        nc.scalar.dma_start(DP1[:], DP1_d[:])
        E0 = cpool.tile([V, 1], f32)
        nc.gpsimd.dma_start(E0[:], E0_d[:])
        ONESI = cpool.tile([128, 4 * V], i32)
        nc.vector.memset(ONESI[:], 1)

        if repeat:
            ctx.enter_context(tc.For_i(0, repeat, 1))

        # ---------------- setup: per-(n,p) edge constants ----------------
        # POLY0/POLY1 [128, 32]: row P holds polygon n=P%32, vertices (x,y)*16,
        # POLY1 is rolled by one vertex.
        POLY0 = spool.tile([128, 32], f32)
        POLY1 = spool.tile([128, 32], f32)
        nc.sync.dma_start(
            POLY0[:], bass.AP(tensor=polygons, offset=0, ap=[[0, 4], [32, 32], [1, 32]]))
        nc.sync.dma_start(
            POLY1[:, 0:30], bass.AP(tensor=polygons, offset=2, ap=[[0, 4], [32, 32], [1, 30]]))
        nc.sync.dma_start(
            POLY1[:, 30:32], bass.AP(tensor=polygons, offset=0, ap=[[0, 4], [32, 32], [1, 2]]))

        x0 = POLY0[:, 0:32:2]
        y0 = POLY0[:, 1:32:2]
        x1 = POLY1[:, 0:32:2]
        y1 = POLY1[:, 1:32:2]

        EX = spool.tile([128, P], f32)
        nc.vector.tensor_tensor(EX[:], x1, x0, A.subtract)
        EY = spool.tile([128, P], f32)
        nc.vector.tensor_tensor(EY[:], y1, y0, A.subtract)
        EX2 = spool.tile([128, P], f32)
        nc.vector.tensor_tensor(EX2[:], EX[:], EX[:], A.mult)
        EY2 = spool.tile([128, P], f32)
        nc.vector.tensor_tensor(EY2[:], EY[:], EY[:], A.mult)
        ESQ = spool.tile([128, P], f32)
        nc.vector.tensor_tensor(ESQ[:], EX2[:], EY2[:], A.add)
        nc.vector.tensor_scalar(ESQ[:], ESQ[:], EPS, None, A.add)
        L = spool.tile([128, P], f32)
        nc.scalar.activation(L[:], ESQ[:], AF.Sqrt)
        RS = spool.tile([128, P], f32)
        nc.vector.reciprocal(RS[:], L[:])
        EXQ = spool.tile([128, P], f32)
        nc.vector.tensor_tensor(EXQ[:], EX[:], RS[:], A.mult)
        EYQ = spool.tile([128, P], f32)
        nc.vector.tensor_tensor(EYQ[:], EY[:], RS[:], A.mult)
        T0 = spool.tile([128, P], f32)
        nc.vector.tensor_tensor(T0[:], x0, EXQ[:], A.mult)
        T1 = spool.tile([128, P], f32)
        nc.vector.tensor_tensor(T1[:], y0, EYQ[:], A.mult)
        D0Q = spool.tile([128, P], f32)
        nc.vector.tensor_tensor(D0Q[:], T0[:], T1[:], A.add)
        nc.vector.tensor_scalar(D0Q[:], D0Q[:], -1.0, None, A.mult)

        # polygon validity: (validity >= 0.5) & (count(x+y != 0) >= 3)
        SXY = spool.tile([128, P], f32)
        nc.vector.tensor_tensor(SXY[:], x0, y0, A.add)
        NZ = spool.tile([128, P], f32)
        nc.vector.tensor_scalar(NZ[:], SXY[:], 0.0, None, A.not_equal)
        CNT = spool.tile([128, 1], f32)
        nc.vector.tensor_reduce(CNT[:], NZ[:], axis=mybir.AxisListType.X, op=A.add)
        OKC = spool.tile([128, 1], f32)
        nc.vector.tensor_scalar(OKC[:], CNT[:], 3.0, None, A.is_ge)
        VV = spool.tile([128, 1], f32)
        nc.sync.dma_start(
            VV[:], bass.AP(tensor=validity, offset=0, ap=[[0, 4], [1, 32], [0, 1]]))
        OKV = spool.tile([128, 1], f32)
        nc.vector.tensor_scalar(OKV[:], VV[:], 0.5, None, A.is_ge)
        OK = spool.tile([128, 1], f32)
        nc.vector.tensor_tensor(OK[:], OKC[:], OKV[:], A.mult)

        # height mask dm[d] = (d < clip(floor(64*attr0),1,64))
        A6 = spool.tile([1, 6], f32)
        nc.sync.dma_start(A6[:], attributes[:].unsqueeze(0))
        A0B = spool.tile([V, 1], f32)
        nc.gpsimd.partition_broadcast(A0B[:], A6[0:1, 0:1], channels=V)
        F64 = spool.tile([V, 1], f32)
        nc.vector.tensor_scalar(F64[:], A0B[:], 64.0, None, A.mult)
        DM = spool.tile([V, 1], f32)
        nc.vector.tensor_tensor(DM[:], F64[:], DP1[:], A.is_ge)
        nc.vector.tensor_tensor(DM[:], DM[:], E0[:], A.max)

        # ---------------- crossing: parity words ----------------
        # layout: partitions = (p_local*32 + n) for p = c*4 + p_local
        ZM = xpool.tile([128, 4 * V], f32)        # 63*inter_x, yc-gated, c-major
        pgX = polygons[:].rearrange("n p t -> p n t")
        for c in range(4):
            PX0 = xpool.tile([128, 2], f32, tag="px0")
            nc.sync.dma_start(PX0[:], pgX[c * 4:(c + 1) * 4, :, :])
            PX1 = xpool.tile([128, 2], f32, tag="px1")
            if c < 3:
                nc.sync.dma_start(PX1[:], pgX[c * 4 + 1:(c + 1) * 4 + 1, :, :])
            else:
                nc.sync.dma_start(PX1[0:96, :], pgX[13:16, :, :])
                nc.sync.dma_start(PX1[96:128, :], pgX[0:1, :, :])
            cx0 = PX0[:, 0:1]
            cy0 = PX0[:, 1:2]
            cx1 = PX1[:, 0:1]
            cy1 = PX1[:, 1:2]
            DY = xpool.tile([128, 1], f32, tag="dy")
            nc.vector.tensor_tensor(DY[:], cy1, cy0, A.subtract)
            nc.vector.tensor_scalar(DY[:], DY[:], EPS, None, A.add)
            RD = xpool.tile([128, 1], f32, tag="rd")
            nc.vector.reciprocal(RD[:], DY[:])
            X063 = xpool.tile([128, 1], f32, tag="x063")
            nc.vector.tensor_scalar(X063[:], cx0, 63.0, None, A.mult)
            DX63 = xpool.tile([128, 1], f32, tag="dx63")
            nc.vector.scalar_tensor_tensor(DX63[:], cx1, 63.0, X063[:], A.mult, A.subtract)

            RAT = xpool.tile([128, V], f32, tag="rat")
            nc.vector.tensor_scalar(RAT[:], YG[:], cy0, RD[:], A.subtract, A.mult)
            ZC = xpool.tile([128, V], f32, tag="zc")
            nc.vector.tensor_scalar(ZC[:], RAT[:], DX63[:], X063[:], A.mult, A.add)
            YCA = xpool.tile([128, V], f32, tag="yca")
            nc.vector.tensor_scalar(YCA[:], YG[:], cy0, None, A.is_ge)
            YC = xpool.tile([128, V], f32, tag="yc")
            nc.vector.scalar_tensor_tensor(YC[:], YG[:], cy1, YCA[:], A.is_ge, A.not_equal)
            nc.vector.tensor_tensor(ZM[:, c * V:(c + 1) * V], ZC[:], YC[:], A.mult)

        # K = number of grid x-columns strictly left of the intersection
        # K = ceil(z) computed cast-mode-agnostically: ki = cast(z) (trunc OR
        # rint both fine), K = ki + (z > float(ki)).
        ZCl = xpool.tile([128, 4 * V], f32)
        nc.vector.tensor_scalar(ZCl[:], ZM[:], -1.0, 65.0, A.max, A.min)
        KI = xpool.tile([128, 4 * V], i32)
        nc.vector.tensor_copy(KI[:], ZCl[:])
        KF = xpool.tile([128, 4 * V], f32)
        nc.vector.tensor_copy(KF[:], KI[:])
        GT = xpool.tile([128, 4 * V], i32)
        nc.vector.tensor_tensor(GT[:], ZCl[:], KF[:], A.is_gt)
        K = xpool.tile([128, 4 * V], i32)
        nc.vector.tensor_tensor(K[:], KI[:], GT[:], A.add)
        # masks as 4 x 16-bit words (values <= 65535: exact on any ALU path,
        # shifts <= 16: no shift-overflow ambiguity): mw = (1 << clamp(K-16w,
        # 0,16)) - 1
        MW = []
        for w in range(4):
            MWt = xpool.tile([128, 4 * V], i32, tag=f"mw{w}")
            MW.append(MWt)
        KT = xpool.tile([128, 4 * V], i32, tag="kt")
        KW = xpool.tile([128, 4 * V], i32, tag="kw")
        SHW = xpool.tile([128, 4 * V], i32, tag="shw")
        for w in range(4):
            nc.vector.tensor_scalar(KT[:], K[:], 16 * w + 16, None, A.min)
            nc.vector.tensor_scalar(KW[:], KT[:], -16 * w, 0, A.add, A.max)
            nc.vector.tensor_tensor(SHW[:], ONESI[:], KW[:], A.logical_shift_left)
            nc.vector.tensor_scalar(MW[w][:], SHW[:], 1, None, A.subtract)

        # fold over c (xor of the four 64-col slices) -> PWpre [128, (w4,y64)]
        PWpre = xpool.tile([128, 4 * V], i32)
        TA = xpool.tile([128, V], i32, tag="ta")
        TB = xpool.tile([128, V], i32, tag="tb")
        for w in range(4):
            nc.vector.tensor_tensor(TA[:], MW[w][:, 0:V], MW[w][:, V:2 * V], A.bitwise_xor)
            nc.vector.tensor_tensor(TB[:], MW[w][:, 2 * V:3 * V], MW[w][:, 3 * V:4 * V], A.bitwise_xor)
            nc.vector.tensor_tensor(
                PWpre[:].rearrange("p (y w) -> p y w", w=4)[:, :, w],
                TA[:], TB[:], A.bitwise_xor)

        # fold over p_local via DRAM bounce: [128=(p4,n32), 256] -> [32, (p4,256)]
        nc.sync.dma_start(pw1_d[:], PWpre[:])
        X2 = xpool.tile([32, 4 * 4 * V], i32)
        nc.sync.dma_start(X2[:], pw1_d[:].rearrange("(p n) q -> n p q", p=4))
        XA = xpool.tile([32, 4 * V], i32)
        nc.vector.tensor_tensor(XA[:], X2[:, 0:256], X2[:, 256:512], A.bitwise_xor)
        XBt = xpool.tile([32, 4 * V], i32)
        nc.vector.tensor_tensor(XBt[:], X2[:, 512:768], X2[:, 768:1024], A.bitwise_xor)
        PW = xpool.tile([32, 4 * V], i32)
        nc.vector.tensor_tensor(PW[:], XA[:], XBt[:], A.bitwise_xor)
        nc.sync.dma_start(pw_d[:].rearrange("n y w -> n (y w)"), PW[:])

        # compact parity words per partition: WN[P=(mb,n), (fy,w)] = pw[n, mb*16+fy, w]
        WN = fpool.tile([128, V], i32)
        for mb in range(4):
            srcap = bass.AP(tensor=pw_d, offset=mb * 64,
                            ap=[[4 * V, 32], [4, 16], [1, 4]])
            nc.sync.dma_start(WN[mb * 32:(mb + 1) * 32, :], srcap)

        IB = fpool.tile([128, F], i32, tag="fC")
        WNb = (WN[:].rearrange("p (fy w) -> p fy w", fy=16)
               .unsqueeze(3).to_broadcast([128, 16, 4, 16]))
        nc.vector.tensor_tensor(
            IB[:].rearrange("p (fy w x) -> p fy w x", fy=16, w=4),
            WNb,
            XB[:].rearrange("p (fy w x) -> p fy w x", fy=16, w=4),
            A.bitwise_and)

        # ---------------- distance: p-loop with min accumulation ----------------
        ACC = apool.tile([128, F], f32)
        for p in range(P):
            exq = EXQ[:, p:p + 1]
            eyq = EYQ[:, p:p + 1]
            Tt = dpool.tile([128, F], f32, tag="T")
            nc.scalar.activation(Tt[:], PY[:], AF.Identity, bias=D0Q[:, p:p + 1], scale=eyq)
            Q = dpool.tile([128, F], f32, tag="Q")
            nc.vector.scalar_tensor_tensor(Q[:], PX[:], exq, Tt[:], A.mult, A.add)
            W = Q
            nc.vector.tensor_scalar(W[:], Q[:], 0.0, L[:, p:p + 1], A.max, A.min)
            GX = dpool.tile([128, F], f32, tag="GX")
            nc.vector.scalar_tensor_tensor(GX[:], W[:], exq, PX[:], A.mult, A.subtract)
            GY = dpool.tile([128, F], f32, tag="GY")
            nc.vector.scalar_tensor_tensor(GY[:], W[:], eyq, PY[:], A.mult, A.subtract)
            SX = dpool.tile([128, F], f32, tag="SX")
            nc.scalar.activation(SX[:], GX[:], AF.Square, bias=x0[:, p:p + 1])
            SY = dpool.tile([128, F], f32, tag="SY")
            nc.scalar.activation(SY[:], GY[:], AF.Square, bias=y0[:, p:p + 1])
            zeng = nc.gpsimd if Z_ENGINE == "gpsimd" else nc.vector
            if p == 0:
                zeng.tensor_tensor(ACC[:], SX[:], SY[:], A.add)
            else:
                Z = dpool.tile([128, F], f32, tag="Z")
                zeng.tensor_tensor(Z[:], SX[:], SY[:], A.add)
                nc.vector.tensor_tensor(ACC[:], ACC[:], Z[:], A.min)

        # ---------------- final combine ----------------
        D = fpool.tile([128, F], f32, tag="fA")
        nc.scalar.activation(D[:], ACC[:], AF.Sqrt)
        SO = fpool.tile([128, F], f32, tag="fB")
        nc.scalar.activation(SO[:], D[:], AF.Sigmoid, scale=-SHARP)
        DF = fpool.tile([128, F], f32, tag="fA")
        nc.vector.scalar_tensor_tensor(DF[:], IB[:], 0.0, SO[:], A.not_equal, A.subtract)
        MG = fpool.tile([128, F], f32, tag="fB")
        nc.scalar.activation(MG[:], DF[:], AF.Abs, scale=OK[:])

        # max over n, chunked by mb: [32 rows=n, 1024] -> all-reduce -> comb
        for mb in range(4):
            M2 = fpool.tile([32, F], f32, tag=f"m{mb % 2}")
            nc.scalar.dma_start(M2[:], MG[mb * 32:(mb + 1) * 32, :])
            CMB = fpool.tile([32, F], f32, tag=f"c{mb % 2}")
            nc.gpsimd.partition_all_reduce(CMB[:], M2[:], channels=32, reduce_op=RO.max)
            nc.scalar.dma_start(comb_d[mb * F:(mb + 1) * F].unsqueeze(0), CMB[0:1, :])

        # extrude: CB[d, m] = comb[m] * dm[d]
        for hh in range(2):
            CB = fpool.tile([V, M // 2], f32, tag=f"cb{hh}")
            nc.sync.dma_start(
                CB[:], bass.AP(tensor=comb_d, offset=hh * (M // 2),
                               ap=[[0, V], [1, M // 2]]))
            nc.vector.tensor_scalar(CB[:], CB[:], DM[:], None, A.mult)
            nc.sync.dma_start(
                out[:].rearrange("d h w -> d (h w)")[:, hh * (M // 2):(hh + 1) * (M // 2)],
                CB[:])

    nc.compile()
    return nc


_cached = None


def _get_program():
    global _cached
    if _cached is None:
        _cached = _build_program()
    return _cached


def kernel(polygons: np.ndarray, attributes: np.ndarray,
           validity_scores: np.ndarray) -> np.ndarray:
    nc = _get_program()
    B = polygons.shape[0]
    in_maps = [
        {
            "polygons": np.ascontiguousarray(polygons[b], dtype=np.float32),
            "attributes": np.ascontiguousarray(attributes[b], dtype=np.float32),
            "validity_scores": np.ascontiguousarray(validity_scores[b], dtype=np.float32),
        }
        for b in range(B)
    ]
    res = run_bass_kernel_spmd(nc, in_maps, core_ids=list(range(B)))
    return np.stack([res.results[b]["out"] for b in range(B)], axis=0)



# revision 12
# speedup vs baseline: 2.7130x; 2.7130x over previous
"""Trainium2 Bass kernel for DifferentiableExtrusion.

Takes FULL inputs (B=8 batch), shards batch across 8 NeuronCores (1 b per
core), runs a Tile-scheduled Bass kernel per core, gathers full output.

Per-core algorithm (V=64 grid, N=32 polygons, P=16 vertices):
  distance: point-segment distance via dist^2 = Cq^2 + B^2 where
    Q  = dot(pt - v0, e_hat)   (arc-length projection, affine in pt)
    Cq = cross(pt - v0, e_hat) (perp line distance, affine in pt)
    B  = relu(|Q - L/2| - L/2) (overshoot beyond the segment ends)
    Q comes from a K=3 TensorE matmul (features [px, py, 1]); Cq^2 comes
    DIRECTLY from a K=6 matmul (features [px^2, py^2, px*py, px, py, 1]),
    both f32r at 1 cycle/row.  Elementwise chain is fp16 on DVE with
    engine-assignment knobs (vector/scalar/gpsimd) per unit.
    Layout: partitions = edge (g*32+n), free = grid points; 4 edge-chunks
    x 4 m-quarters; min-folds over chunks then partitions -> [32n, 1024].
  inside test: exact ray-cast parity at (n,p,y) granularity: intersection
    x quantized to a 64-bit column mask (4 int16-in-int32 words), masks
    XOR-reduced over edges via SBUF partition folds -> per-(n,y) parity
    words, broadcast back to the grid and tested against per-column bits.
  combine: mask = |inside - sigmoid(-100*d)|, gated by polygon validity,
    max-reduced over n (gpsimd partition_all_reduce), extruded over D=64
    slices with the per-b height mask, DMA'd out.
"""

import numpy as np
from contextlib import ExitStack

import concourse.bass as bass
import concourse.bacc as bacc
import concourse.mybir as mybir
import concourse.tile as tile
import concourse.bass_isa as bass_isa
from concourse.bass_utils import run_bass_kernel_spmd

f32 = mybir.dt.float32
f32r = mybir.dt.float32r
f16 = mybir.dt.float16
i32 = mybir.dt.int32
A = mybir.AluOpType
AF = mybir.ActivationFunctionType
RO = bass_isa.ReduceOp

V = 64
N = 32
P = 16
M = V * V          # 4096 grid points
F = M // 4         # 1024 points per m-quarter
SHARP = 100.0
EPS = 1e-8

# ---- tuning knobs ----
DT = f16            # dtype of the elementwise distance chain
# per-(mq,c) engine for B2 (square of overshoot): 's'calar | 'v'ector | 'g'psimd
B2_ENG = ["v"] * 16
# engine for the min-accumulate TT: 'v' | 'g'
MIN_ENG = ["v"] * 16
# engine for D2 = Cq2 + B2 (reads PSUM -> vector only unless gpsimd can)
D2_ENG = ["v"] * 16


def _make_consts():
    grid = (np.arange(V, dtype=np.float64) / np.float64(V - 1.0))
    f = np.arange(F)
    x_of_f = f % V                      # x index for free position f
    # F6 features rows: px, py, 1, px^2, py^2, px*py  over m = y*64+x
    # (affine rows first so the K=3 Q-matmul can use base partition 0)
    m = np.arange(M)
    px = grid[m % V]
    py = grid[m // V]
    F6 = np.stack([px, py, np.ones(M), px * px, py * py, px * py], 0)
    F6 = F6.astype(np.float32)
    XB = np.broadcast_to((np.int32(1) << (x_of_f % 16).astype(np.int32)), (128, F)).astype(np.int32)
    YG = np.broadcast_to(grid.astype(np.float32), (128, V)).astype(np.float32)
    DP1 = (np.arange(V, dtype=np.float32) + 1.0).reshape(V, 1).astype(np.float32)
    E0 = np.zeros((V, 1), np.float32)
    E0[0, 0] = 1.0
    return F6, XB, YG, DP1, E0


def _build_program(repeat=None):
    nc = bacc.Bacc("TRN2", target_bir_lowering=False, debug=False)

    polygons = nc.dram_tensor("polygons", [N, P, 2], f32, kind="ExternalInput")
    attributes = nc.dram_tensor("attributes", [6], f32, kind="ExternalInput")
    validity = nc.dram_tensor("validity_scores", [N], f32, kind="ExternalInput")
    out = nc.dram_tensor("out", [V, V, V], f32, kind="ExternalOutput")

    F6n, XBn, YGn, DP1n, E0n = _make_consts()
    F6_d = nc.inline_tensor(F6n, "F6_c")
    XB_d = nc.inline_tensor(XBn, "XB_c")
    YG_d = nc.inline_tensor(YGn, "YG_c")
    DP1_d = nc.inline_tensor(DP1n, "DP1_c")
    E0_d = nc.inline_tensor(E0n, "E0_c")

    comb_d = nc.dram_tensor("comb_scratch", [M], DT)        # combined mask, flat

    with tile.TileContext(nc) as tc, ExitStack() as ctx:
        cpool = ctx.enter_context(tc.tile_pool(name="consts", bufs=1))
        spool = ctx.enter_context(tc.tile_pool(name="setup", bufs=1))
        xpool = ctx.enter_context(tc.tile_pool(name="cross", bufs=1))
        dpool = ctx.enter_context(tc.tile_pool(name="dist", bufs=3))
        apool = ctx.enter_context(tc.tile_pool(name="accum", bufs=1))
        fpool = ctx.enter_context(tc.tile_pool(name="final", bufs=1))
        ppool = ctx.enter_context(tc.psum_pool(name="mm", bufs=2))

        # ---------------- constants into SBUF ----------------
        F6 = cpool.tile([6, M], f32)
        nc.scalar.dma_start(F6[:], F6_d[:])
        XB = cpool.tile([128, F], i32)
        nc.scalar.dma_start(XB[:], XB_d[:])
        YG = cpool.tile([128, V], f32)
        nc.gpsimd.dma_start(YG[:], YG_d[:])
        DP1 = cpool.tile([V, 1], f32)
        nc.scalar.dma_start(DP1[:], DP1_d[:])
        E0 = cpool.tile([V, 1], f32)
        nc.gpsimd.dma_start(E0[:], E0_d[:])
        ONESI = cpool.tile([128, 4 * V], i32)
        nc.vector.memset(ONESI[:], 1)

        if repeat:
            ctx.enter_context(tc.For_i(0, repeat, 1))

        # ---------------- setup: POLY-layout (validity, heights, L col) ----
        # POLY0/POLY1 [128, 32]: row q=(g*32+n) holds polygon n, verts (x,y)*16,
        # POLY1 is rolled by one vertex.
        POLY0 = spool.tile([128, 32], f32)
        POLY1 = spool.tile([128, 32], f32)
        nc.sync.dma_start(
            POLY0[:], bass.AP(tensor=polygons, offset=0, ap=[[0, 4], [32, 32], [1, 32]]))
        nc.sync.dma_start(
            POLY1[:, 0:30], bass.AP(tensor=polygons, offset=2, ap=[[0, 4], [32, 32], [1, 30]]))
        nc.sync.dma_start(
            POLY1[:, 30:32], bass.AP(tensor=polygons, offset=0, ap=[[0, 4], [32, 32], [1, 2]]))

        x0v = POLY0[:, 0:32:2]
        y0v = POLY0[:, 1:32:2]
        x1v = POLY1[:, 0:32:2]
        y1v = POLY1[:, 1:32:2]

        # polygon validity: (validity >= 0.5) & (count(x+y != 0) >= 3)
        SXY = spool.tile([128, P], f32)
        nc.vector.tensor_tensor(SXY[:], x0v, y0v, A.add)
        NZ = spool.tile([128, P], f32)
        nc.vector.tensor_scalar(NZ[:], SXY[:], 0.0, None, A.not_equal)
        CNT = spool.tile([128, 1], f32)
        nc.vector.tensor_reduce(CNT[:], NZ[:], axis=mybir.AxisListType.X, op=A.add)
        OKC = spool.tile([128, 1], f32)
        nc.vector.tensor_scalar(OKC[:], CNT[:], 3.0, None, A.is_ge)
        VV = spool.tile([128, 1], f32)
        nc.sync.dma_start(
            VV[:], bass.AP(tensor=validity, offset=0, ap=[[0, 4], [1, 32], [0, 1]]))
        OKV = spool.tile([128, 1], f32)
        nc.vector.tensor_scalar(OKV[:], VV[:], 0.5, None, A.is_ge)
        OK = spool.tile([128, 1], f32)
        nc.vector.tensor_tensor(OK[:], OKC[:], OKV[:], A.mult)

        # height mask dm[d] = (d < clip(floor(64*attr0),1,64))
        A6 = spool.tile([1, 6], f32)
        nc.sync.dma_start(A6[:], attributes[:].unsqueeze(0))
        A0B = spool.tile([V, 1], f32)
        nc.gpsimd.partition_broadcast(A0B[:], A6[0:1, 0:1], channels=V)
        F64 = spool.tile([V, 1], f32)
        nc.vector.tensor_scalar(F64[:], A0B[:], 64.0, None, A.mult)
        DM = spool.tile([V, 1], f32)
        nc.vector.tensor_tensor(DM[:], F64[:], DP1[:], A.is_ge)
        nc.vector.tensor_tensor(DM[:], DM[:], E0[:], A.max)

        # per-edge length L in POLY layout -> LCOL [128,4] (edge q=(g,n), col c)
        EX16 = spool.tile([128, P], f32)
        nc.vector.tensor_tensor(EX16[:], x1v, x0v, A.subtract)
        EY16 = spool.tile([128, P], f32)
        nc.vector.tensor_tensor(EY16[:], y1v, y0v, A.subtract)
        EXQ16 = spool.tile([128, P], f32)
        nc.vector.tensor_tensor(EXQ16[:], EX16[:], EX16[:], A.mult)
        EYQ16 = spool.tile([128, P], f32)
        nc.vector.tensor_tensor(EYQ16[:], EY16[:], EY16[:], A.mult)
        ESQ16 = spool.tile([128, P], f32)
        nc.vector.tensor_tensor(ESQ16[:], EXQ16[:], EYQ16[:], A.add)
        nc.vector.tensor_scalar(ESQ16[:], ESQ16[:], EPS, None, A.add)
        L16 = spool.tile([128, P], f32)
        nc.scalar.activation(L16[:], ESQ16[:], AF.Sqrt)
        LCOL = spool.tile([128, 4], f32)
        for g in range(4):
            nc.vector.tensor_copy(
                LCOL[g * 32:(g + 1) * 32, :], L16[g * 32:(g + 1) * 32, 4 * g:4 * g + 4])
        LH = spool.tile([128, 4], f32)
        nc.vector.tensor_scalar(LH[:], LCOL[:], 0.5, None, A.mult)
        NLH = spool.tile([128, 4], f32)
        nc.vector.tensor_scalar(NLH[:], LCOL[:], -0.5, None, A.mult)

        # ---------------- setup: free-layout E matrices ----------------
        # edge j = c*128 + g*32 + n  handles polygon-n edge p = 4g + c.
        # XY0/XY1 [1, 1024]: interleaved (x,y) pairs in edge order.
        XY0 = spool.tile([1, 2 * 512], f32)
        XY1 = spool.tile([1, 2 * 512], f32)
        for c in range(4):
            nc.sync.dma_start(
                XY0[:, c * 256:(c + 1) * 256],
                bass.AP(tensor=polygons, offset=2 * c, ap=[[0, 1], [8, 4], [32, 32], [1, 2]]))
        for c in range(3):
            nc.sync.dma_start(
                XY1[:, c * 256:(c + 1) * 256],
                bass.AP(tensor=polygons, offset=2 * (c + 1), ap=[[0, 1], [8, 4], [32, 32], [1, 2]]))
        nc.sync.dma_start(
            XY1[:, 768:960],
            bass.AP(tensor=polygons, offset=8, ap=[[0, 1], [8, 3], [32, 32], [1, 2]]))
        nc.sync.dma_start(
            XY1[:, 960:1024],
            bass.AP(tensor=polygons, offset=0, ap=[[0, 1], [32, 32], [1, 2]]))

        x0r = XY0[:, 0:1024:2]
        y0r = XY0[:, 1:1024:2]
        x1r = XY1[:, 0:1024:2]
        y1r = XY1[:, 1:1024:2]

        EXr = spool.tile([1, 512], f32)
        nc.vector.tensor_tensor(EXr[:], x1r, x0r, A.subtract)
        EYr = spool.tile([1, 512], f32)
        nc.vector.tensor_tensor(EYr[:], y1r, y0r, A.subtract)
        EX2r = spool.tile([1, 512], f32)
        nc.vector.tensor_tensor(EX2r[:], EXr[:], EXr[:], A.mult)
        EY2r = spool.tile([1, 512], f32)
        nc.vector.tensor_tensor(EY2r[:], EYr[:], EYr[:], A.mult)
        ESQr = spool.tile([1, 512], f32)
        nc.vector.tensor_tensor(ESQr[:], EX2r[:], EY2r[:], A.add)
        nc.vector.tensor_scalar(ESQr[:], ESQr[:], EPS, None, A.add)
        Lr = spool.tile([1, 512], f32)
        nc.scalar.activation(Lr[:], ESQr[:], AF.Sqrt)
        RSr = spool.tile([1, 512], f32)
        nc.vector.reciprocal(RSr[:], Lr[:])

        # All 9 E-matrix rows computed as free-slices of one partition-0 row
        # (compute engines can only address partition starts 0/32/64/96),
        # then 2 SBUF->SBUF DMAs redistribute into EQ [3,512] / EC [6,512].
        # Row order: exq, eyq, -d0 | px-, py-, 1-, px^2-, py^2-, pxpy-coeff
        # of Cq^2, matching the F6 feature row order (px, py, 1, px2, py2, pxpy).
        ROW0 = spool.tile([1, 9 * 512], f32)
        exq = ROW0[:, 0 * 512:1 * 512]
        eyq = ROW0[:, 1 * 512:2 * 512]
        nc.vector.tensor_tensor(exq, EXr[:], RSr[:], A.mult)
        nc.vector.tensor_tensor(eyq, EYr[:], RSr[:], A.mult)
        T1r = spool.tile([1, 512], f32)
        nc.vector.tensor_tensor(T1r[:], x0r, exq, A.mult)
        T2r = spool.tile([1, 512], f32)
        nc.vector.tensor_tensor(T2r[:], y0r, eyq, A.mult)
        nc.vector.scalar_tensor_tensor(ROW0[:, 2 * 512:3 * 512], T1r[:], -1.0, T2r[:],
                                       A.mult, A.subtract)
        C0r = spool.tile([1, 512], f32)
        T3r = spool.tile([1, 512], f32)
        nc.vector.tensor_tensor(T3r[:], x0r, eyq, A.mult)
        T4r = spool.tile([1, 512], f32)
        nc.vector.tensor_tensor(T4r[:], y0r, exq, A.mult)
        nc.vector.tensor_tensor(C0r[:], T3r[:], T4r[:], A.subtract)
        nc.vector.scalar_tensor_tensor(ROW0[:, 3 * 512:4 * 512], eyq, -2.0, C0r[:], A.mult, A.mult)
        nc.vector.scalar_tensor_tensor(ROW0[:, 4 * 512:5 * 512], exq, 2.0, C0r[:], A.mult, A.mult)
        nc.scalar.activation(ROW0[:, 5 * 512:6 * 512], C0r[:], AF.Square)
        nc.scalar.activation(ROW0[:, 6 * 512:7 * 512], eyq, AF.Square)
        nc.scalar.activation(ROW0[:, 7 * 512:8 * 512], exq, AF.Square)
        nc.vector.scalar_tensor_tensor(ROW0[:, 8 * 512:9 * 512], exq, -2.0, eyq, A.mult, A.mult)

        EQ = spool.tile([3, 512], f32)
        nc.scalar.dma_start(EQ[:], ROW0[:, 0:3 * 512])
        EC = spool.tile([6, 512], f32)
        nc.scalar.dma_start(EC[:], ROW0[:, 3 * 512:9 * 512])

        # ---------------- crossing: parity words ----------------
        # layout: partitions = (p_local*32 + n) for p = c*4 + p_local
        pgX = polygons[:].rearrange("n p t -> p n t")
        CXY0 = xpool.tile([128, 8], f32)     # (c, x/y) columns
        CXY1 = xpool.tile([128, 8], f32)
        for c in range(4):
            nc.sync.dma_start(CXY0[:, 2 * c:2 * c + 2], pgX[c * 4:(c + 1) * 4, :, :])
            if c < 3:
                nc.sync.dma_start(CXY1[:, 2 * c:2 * c + 2], pgX[c * 4 + 1:(c + 1) * 4 + 1, :, :])
            else:
                nc.sync.dma_start(CXY1[0:96, 2 * c:2 * c + 2], pgX[13:16, :, :])
                nc.sync.dma_start(CXY1[96:128, 2 * c:2 * c + 2], pgX[0:1, :, :])
        cx0 = CXY0[:, 0:8:2]   # [128, 4]
        cy0 = CXY0[:, 1:8:2]
        cx1 = CXY1[:, 0:8:2]
        cy1 = CXY1[:, 1:8:2]
        DY4 = xpool.tile([128, 4], f32)
        nc.vector.tensor_tensor(DY4[:], cy1, cy0, A.subtract)
        nc.vector.tensor_scalar(DY4[:], DY4[:], EPS, None, A.add)
        RD4 = xpool.tile([128, 4], f32)
        nc.vector.reciprocal(RD4[:], DY4[:])
        X063 = xpool.tile([128, 4], f32)
        nc.vector.tensor_scalar(X063[:], cx0, 63.0, None, A.mult)
        DX63 = xpool.tile([128, 4], f32)
        nc.vector.scalar_tensor_tensor(DX63[:], cx1, 63.0, X063[:], A.mult, A.subtract)
        BC4 = xpool.tile([128, 4], f32)      # -cy0 * rd
        nc.vector.scalar_tensor_tensor(BC4[:], cy0, -1.0, RD4[:], A.mult, A.mult)

        ZM = xpool.tile([128, 4 * V], f32)   # 63*inter_x, yc-gated, c-major
        for c in range(4):
            RAT = xpool.tile([128, V], f32, tag="rat")
            nc.scalar.activation(RAT[:], YG[:], AF.Identity,
                                 bias=BC4[:, c:c + 1], scale=RD4[:, c:c + 1])
            ZC = xpool.tile([128, V], f32, tag="zc")
            nc.scalar.activation(ZC[:], RAT[:], AF.Identity,
                                 bias=X063[:, c:c + 1], scale=DX63[:, c:c + 1])
            YCA = xpool.tile([128, V], f32, tag="yca")
            nc.vector.tensor_scalar(YCA[:], YG[:], cy0[:, c:c + 1], None, A.is_ge)
            YC = xpool.tile([128, V], f32, tag="yc")
            nc.vector.scalar_tensor_tensor(YC[:], YG[:], cy1[:, c:c + 1], YCA[:], A.is_ge, A.not_equal)
            nc.vector.tensor_tensor(ZM[:, c * V:(c + 1) * V], ZC[:], YC[:], A.mult)

        # K = number of grid x-columns strictly left of the intersection
        # K = ceil(z) computed cast-mode-agnostically: ki = cast(z) (trunc OR
        # rint both fine), K = ki + (z > float(ki)).
        ZCl = xpool.tile([128, 4 * V], f32)
        nc.vector.tensor_scalar(ZCl[:], ZM[:], -1.0, 65.0, A.max, A.min)
        KI = xpool.tile([128, 4 * V], i32)
        nc.vector.tensor_copy(KI[:], ZCl[:])
        KF = xpool.tile([128, 4 * V], f32)
        nc.vector.tensor_copy(KF[:], KI[:])
        GT = xpool.tile([128, 4 * V], i32)
        nc.vector.tensor_tensor(GT[:], ZCl[:], KF[:], A.is_gt)
        K = xpool.tile([128, 4 * V], i32)
        nc.vector.tensor_tensor(K[:], KI[:], GT[:], A.add)
        # masks as 4 x 16-bit words (values <= 65535: exact on any ALU path,
        # shifts <= 16: no shift-overflow ambiguity): mw = (1 << clamp(K-16w,
        # 0,16)) - 1
        MW = []
        for w in range(4):
            MWt = xpool.tile([128, 4 * V], i32, tag=f"mw{w}")
            MW.append(MWt)
        KT = xpool.tile([128, 4 * V], i32, tag="kt")
        KW = xpool.tile([128, 4 * V], i32, tag="kw")
        SHW = xpool.tile([128, 4 * V], i32, tag="shw")
        for w in range(4):
            nc.vector.tensor_scalar(KT[:], K[:], 16 * w + 16, None, A.min)
            nc.vector.tensor_scalar(KW[:], KT[:], -16 * w, 0, A.add, A.max)
            nc.vector.tensor_tensor(SHW[:], ONESI[:], KW[:], A.logical_shift_left)
            nc.vector.tensor_scalar(MW[w][:], SHW[:], 1, None, A.subtract)

        # fold over c (xor of the four 64-col slices) -> PWpre [128, (y64,w4)]
        PWpre = xpool.tile([128, 4 * V], i32)
        TA = xpool.tile([128, V], i32, tag="ta")
        TB = xpool.tile([128, V], i32, tag="tb")
        for w in range(4):
            nc.vector.tensor_tensor(TA[:], MW[w][:, 0:V], MW[w][:, V:2 * V], A.bitwise_xor)
            nc.vector.tensor_tensor(TB[:], MW[w][:, 2 * V:3 * V], MW[w][:, 3 * V:4 * V], A.bitwise_xor)
            nc.vector.tensor_tensor(
                PWpre[:].rearrange("p (y w) -> p y w", w=4)[:, :, w],
                TA[:], TB[:], A.bitwise_xor)

        # fold over p_local via SBUF partition folds: [128=(p4,n32), 256] -> [32, 256]
        # (TT needs equal input base partitions -> copy-shift the upper half)
        PS1 = xpool.tile([64, 4 * V], i32)
        nc.vector.tensor_copy(PS1[:], PWpre[64:128, :])
        PF1 = xpool.tile([64, 4 * V], i32)
        nc.vector.tensor_tensor(PF1[:], PWpre[0:64, :], PS1[:], A.bitwise_xor)
        PS2 = xpool.tile([32, 4 * V], i32)
        nc.vector.tensor_copy(PS2[:], PF1[32:64, :])
        PW = xpool.tile([32, 4 * V], i32)
        nc.vector.tensor_tensor(PW[:], PF1[0:32, :], PS2[:], A.bitwise_xor)

        # compact parity words per partition: WN[q=(mb,n), (fy,w)] = PW[n, (mb*16+fy)*4+w]
        WN = fpool.tile([128, V], i32)
        for mb in range(4):
            nc.scalar.dma_start(WN[mb * 32:(mb + 1) * 32, :], PW[:, mb * V:(mb + 1) * V])

        IB = fpool.tile([128, F], i32, tag="fC")
        WNb = (WN[:].rearrange("p (fy w) -> p fy w", fy=16)
               .unsqueeze(3).to_broadcast([128, 16, 4, 16]))
        nc.vector.tensor_tensor(
            IB[:].rearrange("p (fy w x) -> p fy w x", fy=16, w=4),
            WNb,
            XB[:].rearrange("p (fy w x) -> p fy w x", fy=16, w=4),
            A.bitwise_and)

        # ---------------- distance: matmul + elementwise chain ----------------
        FACC = apool.tile([128, F], DT)
        for mq in range(4):
            ACCq = apool.tile([128, F], DT, tag=f"acc{mq}")
            for c in range(4):
                u = mq * 4 + c
                Qp = ppool.tile([128, F], f32, tag="qp")
                Cp = ppool.tile([128, F], f32, tag="cp")
                eq = EQ[:, c * 128:(c + 1) * 128].bitcast(f32r)
                ec = EC[:, c * 128:(c + 1) * 128].bitcast(f32r)
                for h in range(2):
                    rq = F6[0:3, mq * F + h * 512: mq * F + (h + 1) * 512].bitcast(f32r)
                    rc = F6[:, mq * F + h * 512: mq * F + (h + 1) * 512].bitcast(f32r)
                    nc.tensor.matmul(Qp[:, h * 512:(h + 1) * 512], eq, rq, start=True, stop=True)
                    nc.tensor.matmul(Cp[:, h * 512:(h + 1) * 512], ec, rc, start=True, stop=True)
                # A = |Q - L/2| ; B = relu(A - L/2) ; B2 = B^2
                A_ = dpool.tile([128, F], DT, tag="A")
                nc.scalar.activation(A_[:], Qp[:], AF.Abs, bias=NLH[:, c:c + 1])
                B_ = dpool.tile([128, F], DT, tag="B")
                nc.vector.tensor_scalar(B_[:], A_[:], LH[:, c:c + 1], 0.0,
                                        A.subtract, A.max)
                B2 = dpool.tile([128, F], DT, tag="B2")
                be = B2_ENG[u]
                if be == "s":
                    nc.scalar.activation(B2[:], B_[:], AF.Square)
                elif be == "v":
                    nc.vector.tensor_tensor(B2[:], B_[:], B_[:], A.mult)
                else:
                    nc.gpsimd.tensor_tensor(B2[:], B_[:], B_[:], A.mult)
                # D2 = Cq^2 + B2 ; ACC = min(ACC, D2)
                if c == 0:
                    nc.vector.tensor_tensor(ACCq[:], Cp[:], B2[:], A.add)
                else:
                    D2 = dpool.tile([128, F], DT, tag="D2")
                    if D2_ENG[u] == "v":
                        nc.vector.tensor_tensor(D2[:], Cp[:], B2[:], A.add)
                    else:
                        nc.gpsimd.tensor_tensor(D2[:], Cp[:], B2[:], A.add)
                    if MIN_ENG[u] == "v":
                        nc.vector.tensor_tensor(ACCq[:], ACCq[:], D2[:], A.min)
                    else:
                        nc.gpsimd.tensor_tensor(ACCq[:], ACCq[:], D2[:], A.min)
            # fold over partitions: min over the 4 g-groups -> [32n, F].
            # 16-bit DVE TT needs equal input base partitions, so shift the
            # upper half down with a tensor_copy first.
            FT1 = dpool.tile([64, F], DT, tag="ft1")
            nc.vector.tensor_copy(FT1[:], ACCq[64:128, :])
            FO1 = dpool.tile([64, F], DT, tag="fo1")
            nc.vector.tensor_tensor(FO1[:], ACCq[0:64, :], FT1[:], A.min)
            FT2 = dpool.tile([32, F], DT, tag="ft2")
            nc.vector.tensor_copy(FT2[:], FO1[32:64, :])
            nc.vector.tensor_tensor(FACC[mq * 32:(mq + 1) * 32, :],
                                    FO1[0:32, :], FT2[:], A.min)

        # ---------------- final combine ----------------
        D = fpool.tile([128, F], f32, tag="fA")
        nc.scalar.activation(D[:], FACC[:], AF.Sqrt)
        SO = fpool.tile([128, F], DT, tag="fB")
        nc.scalar.activation(SO[:], D[:], AF.Sigmoid, scale=-SHARP)
        IN01 = fpool.tile([128, F], DT, tag="fD")
        nc.vector.tensor_scalar(IN01[:], IB[:], 0, None, A.not_equal)
        DF = fpool.tile([128, F], DT, tag="fE")
        nc.vector.tensor_tensor(DF[:], IN01[:], SO[:], A.subtract)
        MG = fpool.tile([128, F], DT, tag="fF")
        nc.scalar.activation(MG[:], DF[:], AF.Abs, scale=OK[:])

        # max over n, chunked by mb: [32 rows=n, 1024] -> all-reduce -> comb
        for mb in range(4):
            M2 = fpool.tile([32, F], DT, tag=f"m{mb % 2}")
            nc.scalar.dma_start(M2[:], MG[mb * 32:(mb + 1) * 32, :])
            CMB = fpool.tile([32, F], DT, tag=f"c{mb % 2}")
            nc.gpsimd.partition_all_reduce(CMB[:], M2[:], channels=32, reduce_op=RO.max)
            nc.scalar.dma_start(comb_d[mb * F:(mb + 1) * F].unsqueeze(0), CMB[0:1, :])

        # extrude: CB[d, m] = comb[m] * dm[d]
        for hh in range(2):
            CB = fpool.tile([V, M // 2], DT, tag=f"cb{hh}")
            nc.sync.dma_start(
                CB[:], bass.AP(tensor=comb_d, offset=hh * (M // 2),
                               ap=[[0, V], [1, M // 2]]))
            CBO = fpool.tile([V, M // 2], f32, tag=f"co{hh}")
            nc.vector.tensor_scalar(CBO[:], CB[:], DM[:], None, A.mult)
            nc.sync.dma_start(
                out[:].rearrange("d h w -> d (h w)")[:, hh * (M // 2):(hh + 1) * (M // 2)],
                CBO[:])

    nc.compile()
    return nc


_cached = None


def _get_program():
    global _cached
    if _cached is None:
        _cached = _build_program()
    return _cached


def kernel(polygons: np.ndarray, attributes: np.ndarray,
           validity_scores: np.ndarray) -> np.ndarray:
    nc = _get_program()
    B = polygons.shape[0]
    in_maps = [
        {
            "polygons": np.ascontiguousarray(polygons[b], dtype=np.float32),
            "attributes": np.ascontiguousarray(attributes[b], dtype=np.float32),
            "validity_scores": np.ascontiguousarray(validity_scores[b], dtype=np.float32),
        }
        for b in range(B)
    ]
    res = run_bass_kernel_spmd(nc, in_maps, core_ids=list(range(B)))
    return np.stack([res.results[b]["out"] for b in range(B)], axis=0)
